# revision 9
# baseline (speedup 1.0000x reference)
"""Multi-head causal attention (b=4, t=2048, d=1024, 16 heads) on 8 TRN2 cores.

Sharding: data parallel over batch (4) x tensor parallel over head halves (2).
Each core computes, for its (batch, head-group):
  QT/KT = (X @ W^T)^T in [d_head, t] layout, V in [t, d_head] layout,
  scores^T = K Q^T per head (keys on partitions, queries free),
  P^T = exp(scores^T * 1/8) with causal 0/1 mask,
  out^T accumulated via (V|1)^T @ P^T  (extra ones column yields the softmax
  denominator in the same matmul), normalized by the reciprocal row,
  then partial = out @ Wo_slice^T.  Host sums the two head-group partials
  and adds bo.

All matmuls run as float32r (TF32-like, ~1e-4 rel err, 4x the fp32 rate).
"""
import sys
sys.path.insert(0, '/opt/trn_rl_repo')
from contextlib import ExitStack

import numpy as np

import concourse.bass as bass
import concourse.mybir as mybir
import concourse.tile as tile
from concourse import bacc
from concourse.bass import ts, ds
from concourse.bass_utils import run_bass_kernel_spmd
from concourse.masks import make_identity

B, T, D, H, DH = 4, 2048, 1024, 16, 64
HG = 2                  # head groups (tensor parallel factor)
HL = H // HG            # 8 local heads
DL = HL * DH            # 512 local head dims
N_CORES = B * HG        # 8
NQB = T // 512          # 4 query blocks of 512
NTC = T // 128          # 16 key chunks of 128
F32 = mybir.dt.float32
F32R = mybir.dt.float32r
AF = mybir.ActivationFunctionType
SM_SCALE = 1.0 / float(np.sqrt(DH))

_CACHE = {}


def _build_nc():
    nc = bacc.Bacc(num_devices=N_CORES)
    x_d = nc.declare_dram_parameter("x", [T, D], F32, isOutput=False)
    wqt_d = nc.declare_dram_parameter("wqt", [D, DL], F32, isOutput=False)
    wkt_d = nc.declare_dram_parameter("wkt", [D, DL], F32, isOutput=False)
    wvt_d = nc.declare_dram_parameter("wvt", [D, DL], F32, isOutput=False)
    wot_d = nc.declare_dram_parameter("wot", [DL, D], F32, isOutput=False)
    bq_d = nc.declare_dram_parameter("bq", [DL, 1], F32, isOutput=False)
    bk_d = nc.declare_dram_parameter("bk", [DL, 1], F32, isOutput=False)
    bv_d = nc.declare_dram_parameter("bv", [1, DL], F32, isOutput=False)
    out_d = nc.declare_dram_parameter("out", [T, D], F32, isOutput=True)

    with tile.TileContext(nc) as tc, ExitStack() as ctx:
        const = ctx.enter_context(tc.tile_pool(name="const", bufs=1))
        qt_p = ctx.enter_context(tc.tile_pool(name="qt", bufs=4))
        kt_p = ctx.enter_context(tc.tile_pool(name="kt", bufs=4))
        v_p = ctx.enter_context(tc.tile_pool(name="v", bufs=16))

        # --- constants ---
        ident = const.tile([128, 128], F32)
        make_identity(nc, ident[:])
        bqt = const.tile([128, 4], F32)
        bkt = const.tile([128, 4], F32)
        for m in range(4):
            nc.sync.dma_start(bqt[:, m:m + 1], bq_d[ts(m, 128), :])
            nc.sync.dma_start(bkt[:, m:m + 1], bk_d[ts(m, 128), :])
        bv_row = const.tile([1, DL], F32)
        nc.sync.dma_start(bv_row[:], bv_d[:])
        bv_bc = const.tile([128, DL], F32)
        nc.gpsimd.partition_broadcast(bv_bc[:], bv_row[0:1, :])

        qt_t = [qt_p.tile([128, T], F32R, tag="qt", name="qt") for _ in range(4)]
        kt_t = [kt_p.tile([128, T], F32R, tag="kt", name="kt") for _ in range(4)]
        v_t = [v_p.tile([128, HL * 65], F32R, tag="v", name="v")
               for _ in range(NTC)]

        # --- phases A+B: X^T via PE transpose (in two d_in halves), then
        #     QT/KT (transposed) and V (natural) projections ---
        with tc.tile_pool(name="xt", bufs=4) as xt_p, \
             tc.tile_pool(name="xin", bufs=2) as xin_p, \
             tc.tile_pool(name="w", bufs=8) as w_p, \
             tc.tile_pool(name="wv", bufs=8) as wv_p, \
             tc.tile_pool(name="tp_ps", bufs=4, space="PSUM") as tp_ps, \
             tc.tile_pool(name="b_ps", bufs=2, space="PSUM") as b_ps:
            for half in range(2):
                xt_h = [xt_p.tile([128, T], F32R, tag="xt", name="xt")
                        for _ in range(4)]
                for ti in range(NTC):
                    xin = xin_p.tile([128, 512], F32, name="xin")
                    nc.sync.dma_start(
                        xin[:], x_d[ts(ti, 128), ds(half * 512, 512)])
                    for kc in range(4):
                        tp = tp_ps.tile([128, 128], F32, name="tp")
                        nc.tensor.transpose(tp[:], xin[:, ts(kc, 128)], ident[:])
                        nc.vector.tensor_copy(xt_h[kc][:, ts(ti, 128)], tp[:])
                for m in range(4):
                    for w_d, dst, bias in ((wqt_d, qt_t, bqt), (wkt_d, kt_t, bkt)):
                        wts = []
                        for kc in range(4):
                            wt = w_p.tile([128, 128], F32R, name="wt")
                            nc.sync.dma_start(
                                wt[:],
                                w_d[ds(half * 512 + kc * 128, 128),
                                    ts(m, 128)].bitcast(F32R))
                            wts.append(wt)
                        for nt in range(4):
                            ps = b_ps.tile([128, 512], F32, name="bps")
                            for kc in range(4):
                                nc.tensor.matmul(ps[:], wts[kc][:],
                                                 xt_h[kc][:, ts(nt, 512)],
                                                 start=(kc == 0), stop=(kc == 3))
                            dslc = dst[m][:, ts(nt, 512)]
                            if half == 0:
                                nc.vector.tensor_scalar_add(
                                    dslc, ps[:], bias[:, m:m + 1])
                            else:
                                nc.vector.tensor_add(dslc, dslc, ps[:])
                wvts = []
                for kc in range(4):
                    wv = wv_p.tile([128, DL], F32R, name="wvt")
                    nc.sync.dma_start(
                        wv[:],
                        wvt_d[ds(half * 512 + kc * 128, 128), :].bitcast(F32R))
                    wvts.append(wv)
                for tt in range(NTC):
                    ps = b_ps.tile([128, 512], F32, name="bps")
                    for kc in range(4):
                        nc.tensor.matmul(ps[:], xt_h[kc][:, ts(tt, 128)],
                                         wvts[kc][:],
                                         start=(kc == 0), stop=(kc == 3))
                    vv = v_t[tt].rearrange("p (h c) -> p h c", h=HL)[:, :, 0:64]
                    psv = ps[:].rearrange("p (h c) -> p h c", h=HL)
                    if half == 0:
                        nc.vector.tensor_add(
                            vv, psv,
                            bv_bc[:].rearrange("p (h c) -> p h c", h=HL))
                    else:
                        nc.vector.tensor_add(vv, vv, psv)
            ones32 = const.tile([128, HL], F32)
            nc.gpsimd.memset(ones32[:], 1.0)
            onesr = const.tile([128, HL], F32R)
            nc.sync.dma_start(onesr[:], ones32[:].bitcast(F32R))
            for tt in range(NTC):
                nc.sync.dma_start(
                    v_t[tt].rearrange("p (h c) -> p h c", h=HL)[:, :, 64:65],
                    onesr[:].rearrange("p (h c) -> p h c", h=HL))

        # --- attention + output projection, per query block ---
        ot_p = ctx.enter_context(tc.tile_pool(name="ot", bufs=8))
        wot_p = ctx.enter_context(tc.tile_pool(name="wot", bufs=4))
        pt_p = ctx.enter_context(tc.tile_pool(name="pt", bufs=3))
        rec_p = ctx.enter_context(tc.tile_pool(name="rec", bufs=4))
        osb_p = ctx.enter_context(tc.tile_pool(name="osb", bufs=3))
        st_ps = ctx.enter_context(tc.tile_pool(name="st_ps", bufs=2, space="PSUM"))
        o_ps = ctx.enter_context(tc.tile_pool(name="o_ps", bufs=1, space="PSUM"))
        f_ps = ctx.enter_context(tc.tile_pool(name="f_ps", bufs=1, space="PSUM"))

        # causal 0/1 masks for the 4 diagonal offsets, layout [t 128, q 512],
        # duplicated in both halves (two heads share one ST psum tile).
        masks = []
        for j in range(4):
            m32 = const.tile([128, 1024], F32, name="m32", tag="m32")
            nc.gpsimd.memset(m32[:], 1.0)
            for hf in range(2):
                # keep (y - x - 128j) >= 0 else 0
                nc.gpsimd.affine_select(
                    out=m32[:, ts(hf, 512)], in_=m32[:, ts(hf, 512)],
                    compare_op=mybir.AluOpType.is_ge, fill=0.0,
                    base=-128 * j, pattern=[[1, 512]], channel_multiplier=-1,
                )
            mask = const.tile([128, 1024], F32R, name=f"mask{j}", tag=f"mask{j}")
            nc.sync.dma_start(mask[:], m32[:].bitcast(F32R))
            masks.append(mask)

        wot_t = [wot_p.tile([128, D], F32R, tag="wot", name="wot")
                 for _ in range(4)]
        for kc in range(4):
            nc.sync.dma_start(wot_t[kc][:], wot_d[ts(kc, 128), :].bitcast(F32R))

        for qb in range(NQB):
            ots = []
            for p in range(4):  # head pairs: heads 2p, 2p+1
                tmax = 4 * qb + 4
                oA = o_ps.tile([65, 512], F32, tag="opsA", name="opsA")
                oB = o_ps.tile([65, 512], F32, tag="opsB", name="opsB")
                for t in range(tmax):
                    st = st_ps.tile([128, 1024], F32, name="st")
                    nc.tensor.matmul(st[:, 0:512],
                                     kt_t[p][0:64, ts(t, 128)],
                                     qt_t[p][0:64, ts(qb, 512)],
                                     start=True, stop=True)
                    nc.tensor.matmul(st[:, 512:1024],
                                     kt_t[p][64:128, ts(t, 128)],
                                     qt_t[p][64:128, ts(qb, 512)],
                                     start=True, stop=True)
                    pt = pt_p.tile([128, 1024], F32R, name="pt")
                    nc.scalar.activation(pt[:], st[:], AF.Exp, scale=SM_SCALE)
                    if t >= 4 * qb:
                        nc.vector.tensor_mul(pt[:], pt[:], masks[t - 4 * qb][:])
                    first, last = (t == 0), (t == tmax - 1)
                    nc.tensor.matmul(oA[:], v_t[t][:, ds(2 * p * 65, 65)],
                                     pt[:, 0:512], start=first, stop=last)
                    nc.tensor.matmul(oB[:], v_t[t][:, ds((2 * p + 1) * 65, 65)],
                                     pt[:, 512:1024], start=first, stop=last)
                # normalize by the ones-column sums and build OT pair tile
                recA_row = rec_p.tile([1, 512], F32, tag="recAr", name="recAr")
                recB_row = rec_p.tile([1, 512], F32, tag="recBr", name="recBr")
                nc.vector.reciprocal(recA_row[:], oA[64:65, :])
                nc.vector.reciprocal(recB_row[:], oB[64:65, :])
                recA = rec_p.tile([64, 512], F32, tag="recA", name="recA")
                recB = rec_p.tile([64, 512], F32, tag="recB", name="recB")
                nc.gpsimd.partition_broadcast(recA[:], recA_row[0:1, :])
                nc.gpsimd.partition_broadcast(recB[:], recB_row[0:1, :])
                ot = ot_p.tile([128, 512], F32R, tag="ot", name="ot")
                nc.vector.tensor_mul(ot[0:64, :], oA[0:64, :], recA[:])
                nc.vector.tensor_mul(ot[64:128, :], oB[0:64, :], recB[:])
                ots.append(ot)
            # output projection for this query block
            for qt_i in range(4):
                for n in range(2):
                    ps = f_ps.tile([128, 512], F32, name="fps")
                    for kc in range(4):
                        nc.tensor.matmul(ps[:], ots[kc][:, ts(qt_i, 128)],
                                         wot_t[kc][:, ts(n, 512)],
                                         start=(kc == 0), stop=(kc == 3))
                    osb = osb_p.tile([128, 512], F32, name="osb")
                    nc.vector.tensor_copy(osb[:], ps[:])
                    nc.sync.dma_start(
                        out_d[ds(qb * 512 + qt_i * 128, 128), ts(n, 512)],
                        osb[:])

    nc.finalize()
    return nc


def _get_nc():
    if "nc" not in _CACHE:
        _CACHE["nc"] = _build_nc()
    return _CACHE["nc"]


def _make_in_maps(X, Wq, bq, Wk, bk, Wv, bv, Wo, bo):
    X = np.ascontiguousarray(np.asarray(X, dtype=np.float32))
    in_maps = []
    for c in range(N_CORES):
        b, hg = c // HG, c % HG
        sl = slice(hg * DL, (hg + 1) * DL)
        in_maps.append({
            "x": X[b],
            "wqt": np.ascontiguousarray(np.asarray(Wq, dtype=np.float32)[sl, :].T),
            "wkt": np.ascontiguousarray(np.asarray(Wk, dtype=np.float32)[sl, :].T),
            "wvt": np.ascontiguousarray(np.asarray(Wv, dtype=np.float32)[sl, :].T),
            "wot": np.ascontiguousarray(np.asarray(Wo, dtype=np.float32)[:, sl].T),
            "bq": np.asarray(bq, dtype=np.float32)[sl].reshape(DL, 1).copy(),
            "bk": np.asarray(bk, dtype=np.float32)[sl].reshape(DL, 1).copy(),
            "bv": np.asarray(bv, dtype=np.float32)[sl].reshape(1, DL).copy(),
        })
    return in_maps


def kernel(X, Wq, bq, Wk, bk, Wv, bv, Wo, bo):
    nc = _get_nc()
    in_maps = _make_in_maps(X, Wq, bq, Wk, bk, Wv, bv, Wo, bo)
    res = run_bass_kernel_spmd(nc, in_maps, list(range(N_CORES))).results
    bo_np = np.asarray(bo, dtype=np.float32)
    out = np.empty((B, T, D), dtype=np.float32)
    for b in range(B):
        out[b] = res[2 * b]["out"] + res[2 * b + 1]["out"] + bo_np
    return out


if __name__ == "__main__":
    rng = np.random.default_rng(0)
    inputs = {
        "X": rng.standard_normal((B, T, D), dtype=np.float32),
        "Wq": rng.standard_normal((D, D), dtype=np.float32) / 32,
        "bq": np.zeros(D, np.float32),
        "Wk": rng.standard_normal((D, D), dtype=np.float32) / 32,
        "bk": np.zeros(D, np.float32),
        "Wv": rng.standard_normal((D, D), dtype=np.float32) / 32,
        "bv": np.zeros(D, np.float32),
        "Wo": rng.standard_normal((D, D), dtype=np.float32) / 32,
        "bo": np.zeros(D, np.float32),
    }
    out = kernel(**inputs)
    print("kernel ran, out shape", out.shape)
